# revision 45
# baseline (speedup 1.0000x reference)
"""Trainium2 Bass kernel for nn_Encoder_6262062318121 (topk_masking).

Data-parallel over the document axis S=8: one doc per NeuronCore.
Index-dependent gathers and pointwise pooling of gathered rows (mention
mean over M), plus weight-only preprocessing (layout, bf16 casts,
folding the doc-independent rel-attention query v = Wk.T (Wq@rel_cls
+ bq)) happen host-side; all per-document arithmetic runs on-device.

Shapes (per doc): L=512, D=768, H=12, E=32, M=3, R=64, K=51.

Layout notes:
- attg [128, 3, L] bf16: row g = 32h+e = 128t+p holds mean_m
  att[h, starts[e, m], :]  (tile t, partition p).
- DMA rides 3 queues (sync/gpsimd/scalar); one SBUF tile per DMA chunk
  (chunks into one tile share a semaphore and serialize); critical
  tensors (seq, ghp/attg, seqg, vrow) load ahead of the weight halves.
- rel-attention logits are DVE dot products against a broadcast v (no
  seqT layout needed); the softmax row is rebuilt with PE transposes.
- top-k zap chain in f32 (MAX8/MATCH_REPLACE8 are dtype-fixed-cost);
  rescore folds the l1 scales into one scalar per row (c64 = s64/esum).
- output stored fp16, upcast on host.
"""

import numpy as np
import ml_dtypes

import concourse.bacc as bacc
import concourse.mybir as mybir
import concourse.tile as tile
from concourse.bass_utils import run_bass_kernel_spmd

S, L, D, H, E, M, R = 8, 512, 768, 12, 32, 3, 64
KP = 10
K = L * KP // 100  # 51
NCORES = 8
F32 = mybir.dt.float32
BF16 = mybir.dt.bfloat16
F16 = mybir.dt.float16
BF = ml_dtypes.bfloat16

_NROUNDS = (K - 1) // 8  # 6 full zap rounds (48 values)
_THR_COL = K - _NROUNDS * 8 - 1  # index 2 -> 51st largest

AF = mybir.ActivationFunctionType
OP = mybir.AluOpType

# packed-small-tensor column offsets: foldT | ghE | eye128 | vcol
_PK_FOLD = 0
_PK_GHE = 64
_PK_EYE = 192
_PK_VCOL = 320
_PK_W = 326


def _emit(nc, tc, ctx):
    dt = F32
    bf = BF16

    # ---- DRAM parameters (per-core values supplied via in_maps) ----
    # ga: ghp (4*128 cols) | attg tiles (3*512 cols); loaded as 3 chunks
    # into separate tiles so the per-tile DMA semaphores don't serialize
    d_ga = nc.dram_tensor("ga", [128, 2048], bf, kind="ExternalInput").ap()
    d_pk = nc.dram_tensor("pk", [128, _PK_W], bf, kind="ExternalInput").ap()
    d_bh = nc.dram_tensor("bhr", [1, D], bf, kind="ExternalInput").ap()
    d_bt = nc.dram_tensor("btr", [1, D], bf, kind="ExternalInput").ap()
    d_vr = nc.dram_tensor("vrow", [1, D], bf, kind="ExternalInput").ap()
    d_seq = nc.dram_tensor("seq", [128, 4, D], bf, kind="ExternalInput").ap()
    d_seqg = nc.dram_tensor("seqg", [E, M, D], dt, kind="ExternalInput").ap()
    d_wh = nc.dram_tensor("whT", [128, 12, D], bf, kind="ExternalInput").ap()
    d_wt = nc.dram_tensor("wtT", [128, 12, D], bf, kind="ExternalInput").ap()
    d_out = nc.dram_tensor("out", [R, 4, 384], F16, kind="ExternalOutput").ap()

    scale = float(np.float32(1.0) / np.sqrt(np.float32(D)))

    p = ctx.enter_context(tc.tile_pool(name="main", bufs=1))
    pp = ctx.enter_context(tc.tile_pool(name="psum", bufs=1, space="PSUM"))

    # =====================================================================
    # DMA. 3 queues; per-queue trigger order == data arrival order.
    #  sync  : pk, b2, seqT x3, whA x3 (kc0-5), whB x2 (kc6-9)
    #  gpsimd: ga x3 (ghp+attg0 | attg1 | attg2), wtA x3, wtB x2 (kc6-9)
    #  scalar: seqg, seq x2
    # =====================================================================
    seqc = []
    for c in range(2):
        tt = p.tile([128, 2, D], bf, name=f"seq{c}")
        nc.sync.dma_start(out=tt, in_=d_seq[:, 2 * c:2 * c + 2, :])
        seqc.append(tt)
    sb_pk = p.tile([128, _PK_W], bf, name="sb_pk")
    nc.sync.dma_start(out=sb_pk, in_=d_pk)
    sb_bh = p.tile([1, D], bf, name="sb_bh")
    nc.sync.dma_start(out=sb_bh, in_=d_bh)
    sb_bt = p.tile([1, D], bf, name="sb_bt")
    nc.sync.dma_start(out=sb_bt, in_=d_bt)
    wh_a = p.tile([128, 6, D], bf, name="wh_a")
    nc.sync.dma_start(out=wh_a, in_=d_wh[:, 0:6, :])
    wh_b = p.tile([128, 6, D], bf, name="wh_b")
    nc.sync.dma_start(out=wh_b, in_=d_wh[:, 6:12, :])
    wt_b = p.tile([128, 6, D], bf, name="wt_b")
    nc.sync.dma_start(out=wt_b, in_=d_wt[:, 6:12, :])

    sb_ga0 = p.tile([128, 1024], bf, name="sb_ga0")
    nc.gpsimd.dma_start(out=sb_ga0, in_=d_ga[:, 0:1024])
    sb_ga1 = p.tile([128, L], bf, name="sb_ga1")
    nc.gpsimd.dma_start(out=sb_ga1, in_=d_ga[:, 1024:1536])
    sb_ga2 = p.tile([128, L], bf, name="sb_ga2")
    nc.gpsimd.dma_start(out=sb_ga2, in_=d_ga[:, 1536:2048])
    sb_seqg = p.tile([E, M, D], dt, name="sb_seqg")
    nc.gpsimd.dma_start(out=sb_seqg, in_=d_seqg)
    wt_a = p.tile([128, 6, D], bf, name="wt_a")
    nc.gpsimd.dma_start(out=wt_a, in_=d_wt[:, 0:6, :])

    sb_vr = p.tile([1, D], bf, name="sb_vr")
    nc.scalar.dma_start(out=sb_vr, in_=d_vr)

    # views into packed tiles
    ghp = sb_ga0[:, 0:512].rearrange("p (j c) -> p j c", c=128)
    attg = [sb_ga0[:, 512:1024], sb_ga1, sb_ga2]
    foldT = sb_pk[:, _PK_FOLD:_PK_FOLD + 64]
    ghE = sb_pk[0:E, _PK_GHE:_PK_GHE + 128]
    eye128 = sb_pk[:, _PK_EYE:_PK_EYE + 128]
    eye = sb_pk[0:64, _PK_EYE:_PK_EYE + 64]

    ones_bf = p.tile([1, 128], bf, name="ones_bf")
    nc.vector.memset(ones_bf, 1.0)

    # pre-load the Exp table during the DMA front
    warm = p.tile([1, 2], dt, name="warm")
    nc.vector.memset(warm, 1.0)
    nc.scalar.activation(out=warm, in_=warm, func=AF.Exp)

    # =====================================================================
    # PSUM pools: acc (1 bank) + ex (2 bufs x 2 banks) + sm (3 x 1) = 8
    # =====================================================================
    acc_ps = pp.tile([R, L], dt, name="acc_ps", tag="acc", bufs=1)

    # =====================================================================
    # Stage 1 chain (ACT/DVE): ent_emb = ln(sum_m exp(seqg))
    # (exp/ln emitted into the ACT queue interleaved with stage copies)
    # =====================================================================
    seqg_e = p.tile([E, M, D], bf, name="seqg_e")
    se = p.tile([E, D], bf, name="se")
    ent_emb_bf = p.tile([E, D], bf, name="ent_emb_bf")

    # =====================================================================
    # Paired-head expansion: 12 one-hot matmuls; per-pair drain (copy on
    # ACT/DVE/Pool + mul on DVE) keeps the 3-buf sm pool flowing; fold
    # matmuls deferred one pair so the PE never waits on a fresh product.
    # psl (rel-attention logits) interleaves where the PE has slack.
    # =====================================================================
    catHT = p.tile([128, 12, 2 * R], bf, name="catHT")
    copy_eng = [nc.scalar] * 6
    prods = [None] * 6

    # rel-attention logits off the PE: broadcast v to all partitions once,
    # then per-chunk DVE dot products against seq (logT[p,c] = logit(128c+p))
    vrep_ps = pp.tile([128, D], dt, name="vrep_ps", tag="psl", bufs=1)
    nc.tensor.matmul(vrep_ps[:, 0:512], lhsT=ones_bf, rhs=sb_vr[:, 0:512],
                     start=True, stop=True)
    nc.tensor.matmul(vrep_ps[:, 512:768], lhsT=ones_bf, rhs=sb_vr[:, 512:768],
                     start=True, stop=True)
    vrep = p.tile([128, D], bf, name="vrep")
    nc.vector.tensor_copy(vrep, vrep_ps)
    logT = p.tile([128, 4], dt, name="logT")
    ttr_scrap = p.tile([128, D], bf, name="ttr_scrap")

    def emit_pair(k):
        t, j = divmod(k, 2)
        psH = pp.tile([128, L], dt, name=f"psH{k}", tag="sm", bufs=3)
        nc.tensor.matmul(psH, lhsT=ghp[:, j, :], rhs=attg[t],
                         start=True, stop=True)
        psT = pp.tile([128, L], dt, name=f"psT{k}", tag="sm", bufs=3)
        nc.tensor.matmul(psT, lhsT=ghp[:, 2 + j, :], rhs=attg[t],
                         start=True, stop=True)
        sbh = p.tile([128, L], dt, name=f"sbh{k}", tag="sbh", bufs=3)
        eng = copy_eng[k]
        if eng is nc.scalar:
            eng.activation(out=sbh, in_=psH, func=AF.Copy)
        else:
            eng.tensor_copy(sbh, psH)
        prod = p.tile([128, L], bf, name=f"prod{k}", tag="prd", bufs=6)
        nc.vector.tensor_mul(prod, sbh, psT)
        prods[k] = prod

    def emit_fold_add(k):
        # fold matmuls accumulate head pairs + the 128->64 partition fold
        # directly in PSUM: no DVE adds on the a_t critical chain
        nc.tensor.matmul(acc_ps, lhsT=foldT, rhs=prods[k],
                         start=(k == 0), stop=False)

    def emit_logit(c):
        # per-partition dot product via one DVE stt (fills the gaps between
        # the ACT-copy-paced pair muls)
        nc.vector.scalar_tensor_tensor(
            out=ttr_scrap, in0=seqc[c // 2][:, c % 2, :], scalar=1.0,
            in1=vrep, op0=OP.mult, op1=OP.mult,
            accum_out=logT[:, c:c + 1])

    # logits first in the DVE stream: the e-chain's cross-engine latency
    # (exp -> transpose -> broadcast -> relb) overlaps the pair-mul tail
    emit_logit(0)
    emit_logit(1)
    emit_logit(2)
    emit_logit(3)

    emit_pair(0)
    emit_pair(1)
    emit_pair(2)
    emit_fold_add(0)
    emit_pair(3)
    emit_fold_add(1)
    emit_pair(4)

    # softmax numerator: exp over [128, 4], transpose chunks back to a row,
    # broadcast to R rows (interleaved into the pair block so the PE
    # reaches the transposes as soon as e_T is ready)
    e_T = p.tile([128, 4], bf, name="e_T")
    nc.scalar.activation(out=e_T, in_=logT, func=AF.Exp, scale=scale)
    erow_ps = pp.tile([1, L], bf, name="erow_ps", tag="sm", bufs=3)
    for c in range(4):
        nc.tensor.transpose(erow_ps[:, c * 128:(c + 1) * 128],
                            in_=e_T[:, c:c + 1], identity=eye128)
    e_row = p.tile([1, L], bf, name="e_row")
    nc.scalar.copy(e_row, erow_ps)
    psb = pp.tile([R, L], dt, name="psb", tag="psl", bufs=1)
    # (vrep_ps/psb/psoh share one 2-bank ring slot: each is fully drained
    # before the next allocates. PSUM: acc 1 + psl-ring 2 + sm 3 + ex 2 = 8)
    nc.tensor.matmul(psb, lhsT=ones_bf[0:1, 0:R], rhs=e_row,
                     start=True, stop=True)

    emit_fold_add(2)
    emit_pair(5)
    emit_fold_add(3)
    emit_fold_add(4)
    nc.tensor.matmul(acc_ps, lhsT=foldT, rhs=prods[5], start=False, stop=True)

    relb = p.tile([R, L], dt, name="relb")
    es64 = p.tile([R, 1], dt, name="es64")
    nc.scalar.activation(out=relb, in_=psb, func=AF.Copy, accum_out=es64)
    einv64 = p.tile([R, 1], dt, name="einv64")
    nc.vector.reciprocal(einv64, es64)
    # stage-1 chain after relb on ACT (same exp table, no switch); the adds
    # ride the DVE slack just before a_t; Ln's table switch + the tanh warm
    # land in the zap window
    nc.scalar.activation(out=seqg_e, in_=sb_seqg, func=AF.Exp)
    nc.vector.tensor_add(se, seqg_e[:, 0, :], seqg_e[:, 1, :])
    nc.vector.tensor_add(se, se, seqg_e[:, 2, :])
    nc.scalar.activation(out=ent_emb_bf, in_=se, func=AF.Ln)

    # hs/ts one-hot gather in ent space; one ACT drain per chunk
    for dc in range(6):
        ps = pp.tile([128, 2 * R], dt, name=f"ps_hst{dc}", tag="sm", bufs=3)
        nc.tensor.matmul(ps, lhsT=ent_emb_bf[:, dc * 128:(dc + 1) * 128],
                         rhs=ghE, start=True, stop=True)
        nc.scalar.copy(catHT[:, dc, :], ps)

    # extractor accumulators: bias seeds open the PSUM accumulation
    psoh = pp.tile([R, 2, 512], dt, name="psoh", tag="psl", bufs=1)
    psot = pp.tile([R, 2, 512], dt, name="psot", tag="ex", bufs=1)
    for nh in range(2):
        nc.tensor.matmul(psot[:, nh, 0:384], lhsT=ones_bf[0:1, 0:R],
                         rhs=sb_bt[0:1, nh * 384:(nh + 1) * 384],
                         start=True, stop=False)
        nc.tensor.matmul(psoh[:, nh, 0:384], lhsT=ones_bf[0:1, 0:R],
                         rhs=sb_bh[0:1, nh * 384:(nh + 1) * 384],
                         start=True, stop=False)

    # a_t f32 (MAX8/MATCH_REPLACE8 run at fixed cost; bf16 buys nothing)
    a_bf = p.tile([R, L], dt, name="a_bf")
    nc.vector.tensor_mul(a_bf, acc_ps, relb)
    acc_bf = p.tile([R, L], dt, name="acc_bf")
    s64 = p.tile([R, 1], dt, name="s64")
    nc.scalar.activation(out=acc_bf, in_=acc_ps, func=AF.Copy, accum_out=s64)
    nc.scalar.activation(out=warm, in_=warm, func=AF.Tanh)

    c64 = p.tile([R, 1], dt, name="c64")
    nc.vector.tensor_mul(c64, s64, einv64)

    # =====================================================================
    # top-k threshold: bf16 zap chain (exactly 8 replaced per round)
    # =====================================================================
    scr = p.tile([R, L], dt, name="scr")
    m8 = p.tile([R, 8], dt, name="m8")
    cur = a_bf
    for it in range(_NROUNDS):
        nc.vector.max(out=m8, in_=cur)
        nc.vector.match_replace(out=scr, in_to_replace=m8, in_values=cur,
                                imm_value=0.0)
        cur = scr
    nc.vector.max(out=m8, in_=cur)
    thr = m8[:, _THR_COL:_THR_COL + 1]

    # =====================================================================
    # extractor part A (hs/ts halves stream during the zap chain)
    # =====================================================================
    for kc in range(6):
        for nh in range(2):
            nc.tensor.matmul(psot[:, nh, 0:384], lhsT=catHT[:, kc, R:2 * R],
                             rhs=wt_a[:, kc, nh * 384:(nh + 1) * 384],
                             start=False, stop=False)
        for nh in range(2):
            nc.tensor.matmul(psoh[:, nh, 0:384], lhsT=catHT[:, kc, 0:R],
                             rhs=wh_a[:, kc, nh * 384:(nh + 1) * 384],
                             start=False, stop=False)

    # =====================================================================
    # rescore + renormalize (bf16 2x):
    # htu = (a >= thr) * relc + acc ; ht = htu / sum(htu)
    # =====================================================================
    sel2 = p.tile([R, L], dt, name="sel2")
    nc.vector.scalar_tensor_tensor(out=sel2, in0=a_bf, scalar=thr, in1=relb,
                                   op0=OP.is_ge, op1=OP.mult)
    htu = p.tile([R, L], bf, name="htu")
    s2 = p.tile([R, 1], dt, name="s2")
    nc.vector.scalar_tensor_tensor(out=htu, in0=sel2, scalar=c64, in1=acc_bf,
                                   op0=OP.mult, op1=OP.add, accum_out=s2)
    rinv2 = p.tile([R, 1], dt, name="rinv2")
    nc.vector.reciprocal(rinv2, s2)
    # fold the l1 normalization into the transpose identity: the ht
    # transpose is a matmul against eye, so diag(rinv2) applies the
    # per-row scale for free (replaces a full-row tensor_scalar_mul)
    diag_r = p.tile([R, R], bf, name="diag_r")
    nc.vector.tensor_scalar_mul(diag_r, eye, rinv2)
    # keep the PE clock ramped through the rescore wait: the Tensor engine
    # drops out of max pstate after ~a few us idle, which would run the
    # first ~3us of part B at ~1.6x slower clock. These re-folds into the
    # already-drained accumulator (WAR on a_bf/acc_bf puts them in the zap
    # window) are dead work that keeps the pipeline hot.
    for w in range(5):
        nc.tensor.matmul(acc_ps, lhsT=foldT, rhs=prods[5],
                         start=True, stop=True)

    # =====================================================================
    # ht transpose, rs chunks, extractor part B (t first: wt streams on
    # the emptier queues), tanh per nh, fp16 stores split over queues
    # =====================================================================
    # scaled transpose as a plain matmul: out[l, r] = htu[r, l] * rinv2[r]
    ht2T_ps = pp.tile([128, 4, R], dt, name="ht2T_ps", tag="sm", bufs=3)
    for c in range(4):
        nc.tensor.matmul(ht2T_ps[:, c, :],
                         lhsT=htu[:, c * 128:(c + 1) * 128],
                         rhs=diag_r, start=True, stop=True)
    ht2T = p.tile([128, 4, R], bf, name="ht2T")
    nc.vector.tensor_copy(ht2T, ht2T_ps)

    for dc in range(6):
        psr = pp.tile([128, R], dt, name=f"ps_rs{dc}", tag="sm", bufs=3)
        for t in range(4):
            nc.tensor.matmul(psr,
                             lhsT=seqc[t // 2][:, t % 2, dc * 128:(dc + 1) * 128],
                             rhs=ht2T[:, t, :], start=(t == 0), stop=(t == 3))
        # rs is shared between the h and t sides: one bf16 copy serves both
        # (on ACT: the DVE is mid-rescore when these drains are due)
        nc.scalar.copy(catHT[:, 6 + dc, 0:R], psr)

    out_sb = p.tile([R, 4, 384], F16, name="out_sb")
    store_eng = {(0, 0): nc.sync, (0, 1): nc.gpsimd,
                 (1, 0): nc.scalar, (1, 1): nc.sync}
    for side, (wB, pso) in enumerate([(wt_b, psot), (wh_b, psoh)]):
        for kc in range(6, 12):
            w = wB[:, kc - 6, :]
            for nh in range(2):
                nc.tensor.matmul(pso[:, nh, 0:384], lhsT=catHT[:, kc, 0:R],
                                 rhs=w[:, nh * 384:(nh + 1) * 384],
                                 start=False, stop=(kc == 11))
        # side 0 == t half (cols 768:1536), side 1 == h half (cols 0:768)
        ocol = 2 * (1 - side)
        for nh in range(2):
            nc.scalar.activation(out=out_sb[:, ocol + nh, :],
                                 in_=pso[:, nh, 0:384], func=AF.Tanh)
            store_eng[(side, nh)].dma_start(
                out=d_out[:, ocol + nh, :], in_=out_sb[:, ocol + nh, :])


_PROG_CACHE = []


def build_program():
    from contextlib import ExitStack

    if _PROG_CACHE:
        return _PROG_CACHE[0]
    nc = bacc.Bacc("TRN2", target_bir_lowering=False, debug=False)
    with ExitStack() as ctx:
        tc = ctx.enter_context(tile.TileContext(nc))
        _emit(nc, tc, ctx)
    nc.compile()
    _PROG_CACHE.append(nc)
    return nc


def _prep_core(doc, seq_d, att_d, msk_d, starts_d, hts_d, shared):
    """Build the per-core input map (host-side layout/indexing only)."""
    f32 = np.float32
    starts = np.asarray(starts_d).astype(np.int64)  # [E, M]
    hts = np.asarray(hts_d).astype(np.int64)  # [R, 2]

    # attg[p, t, :] = mean_m att[h, starts[e, m], :], g = 128t+p = 32h+e
    g = np.arange(H * E)
    h_of_g, e_of_g = g // E, g % E
    p_of_g, t_of_g = g % 128, g // 128
    rows = att_d[h_of_g[:, None], starts[e_of_g], :]  # [384, M, L]
    attg = np.zeros((128, 3, L), f32)
    attg[p_of_g, t_of_g, :] = rows.mean(axis=1)

    seqg = seq_d[starts.reshape(-1), :].reshape(E, M, D).astype(f32, copy=False)

    # paired-head expansion one-hots: slice j in {0,1} stacks the h-side
    # one-hots of head blocks 2j / 2j+1 in columns 0:64 / 64:128; slices
    # 2+j are the matching t-side one-hots
    r_i = np.arange(R)
    ghp = np.zeros((128, 4, 128), f32)
    for j in range(2):
        for half, a in ((0, 2 * j), (1, 2 * j + 1)):
            ghp[32 * a + hts[:, 0], j, 64 * half + r_i] = 1.0
            ghp[32 * a + hts[:, 1], 2 + j, 64 * half + r_i] = 1.0

    ga = np.concatenate([ghp.reshape(128, 512), attg.reshape(128, 1536)],
                        axis=1)

    ghE = np.zeros((E, 128), f32)
    ghE[hts[:, 0], r_i] = 1.0
    ghE[hts[:, 1], R + r_i] = 1.0
    pk = shared["pk_base"].copy()
    pk[0:E, _PK_GHE:_PK_GHE + 128] = ghE

    seq = np.asarray(seq_d, f32)
    out = {
        "ga": ga.astype(BF),
        "pk": pk.astype(BF),
        "seq": np.ascontiguousarray(
            seq.reshape(4, 128, D).transpose(1, 0, 2).astype(BF)),
        "seqg": np.ascontiguousarray(seqg),
        **shared,
    }
    del out["pk_base"]
    return out


def _shared_inputs(inputs):
    f32 = np.float32
    wq = np.asarray(inputs["Wq"], f32)
    wk = np.asarray(inputs["Wk"], f32)
    bq = np.asarray(inputs["bq"], f32)
    rel = np.asarray(inputs["rel_cls"], f32)
    wh = np.asarray(inputs["Wh"], f32)
    wt = np.asarray(inputs["Wt"], f32)

    # doc-independent rel-attention query, folded host-side:
    # v = Wk.T @ (Wq @ rel + bq); bk only shifts logits (softmax-invariant)
    v = wk.T @ (wq @ rel + bq)

    def chunks(mat, n):  # [n*128, X] -> [128, n, X]
        return np.ascontiguousarray(
            mat.reshape(n, 128, -1).transpose(1, 0, 2).astype(BF))

    whT = chunks(wh.T, 12)
    wtT = chunks(wt.T, 12)

    # ghE gather one-hots are doc-dependent; fill per-core below
    foldT = np.zeros((128, 64), f32)
    r_i = np.arange(R)
    foldT[r_i, r_i] = 1.0
    foldT[R + r_i, r_i] = 1.0

    pk = np.zeros((128, _PK_W), f32)
    pk[:, _PK_FOLD:_PK_FOLD + 64] = foldT
    pk[:, _PK_EYE:_PK_EYE + 128] = np.eye(128, dtype=f32)

    return {
        "pk_base": pk,
        "bhr": np.asarray(inputs["bh"], f32).reshape(1, D).astype(BF),
        "btr": np.asarray(inputs["bt"], f32).reshape(1, D).astype(BF),
        "vrow": v.reshape(1, D).astype(BF),
        "whT": np.ascontiguousarray(whT),
        "wtT": np.ascontiguousarray(wtT),
    }


def kernel(**inputs):
    seq = np.asarray(inputs["sequence_output"], np.float32)  # [S, L, D]
    att = np.asarray(inputs["attention"], np.float32)  # [S, H, L, L]
    msk = np.asarray(inputs["seq_mask"])  # [S, L]
    starts = np.asarray(inputs["mention_starts"])  # [S, E, M]
    hts = np.asarray(inputs["ht_pairs"])  # [S, R, 2]

    shared = _shared_inputs(inputs)
    nc = build_program()
    in_maps = [
        _prep_core(c, seq[c], att[c], msk[c], starts[c], hts[c], shared)
        for c in range(NCORES)
    ]
    res = run_bass_kernel_spmd(nc, in_maps, core_ids=list(range(NCORES)))
    out = np.stack([np.asarray(r["out"], np.float32).reshape(R, 2 * D)
                    for r in res.results])
    return out
